# revision 53
# baseline (speedup 1.0000x reference)
"""Trainium2 Bass kernel for nn_TreeVariationalPosterior (segment_reduce).

Computes, for E=8192 edges and N=4096 nodes:
    B = softmax of branch_logits within each parent group      [E]
    S = segment_sum(B, parent)  (1 for nodes with children)    [N]
    Z = max(0.5 + S[head], eps)                                [E]
    A[i,k] = B[k]/Z[i] where parent[k]==head[i], diag override [E,E]

Sharding: rows of A (edges i) are split across 8 NeuronCores, 1024 rows
per core, built as 8 tiles of [128 rows x 8192 cols].

Key identity used on-device: with ex = exp(logits) (softmax is shift
invariant, so the segment max subtraction is unnecessary for N(0,1)
logits), each output row block is

    A[i,:] = (parent[:]==head[i]) * ex[:] * g[i],
    g[i]   = 1/(ssum[head[i]] * Z[i]),

and ssum[head[i]] = sum_k (parent[k]==head[i]) * ex[k] is exactly the
row-sum of the masked tile.  A single DVE scalar_tensor_tensor per tile
produces both the masked row block AND (via accum_out) its row sums, so
no segment scatter, gather, or cross-core collective is needed.

The in-tile value at the diagonal position (column == global row i) is
(parent[i]==head[i])*ex[i]*g[i], which already equals the reference
diagonal whenever head[i]==parent[i] and is 0 otherwise; the device
emits the remaining (head!=parent) * 0.5/Z[i] correction as a small
per-row vector which the host adds at A[i,i] during unshard assembly.
"""

import numpy as np

E = 8192
N_CORES = 8
ROWS_PER_CORE = E // N_CORES  # 1024
P = 128
TILES = ROWS_PER_CORE // P  # 8


def _ensure_path():
    try:
        import concourse.bass  # noqa: F401
    except ImportError:
        import sys

        for p in ("/opt/trn_rl_repo", "/root/.axon_site/_ro/trn_rl_repo"):
            if p not in sys.path:
                sys.path.insert(0, p)


_PROG = None


def _build_program():
    import concourse.bacc as bacc
    import concourse.mybir as mybir
    from concourse.mybir import AluOpType as alu
    from concourse.tile import TileContext

    fp32 = mybir.dt.float32
    ACT = mybir.ActivationFunctionType

    # Bacc (not plain Bass): its finalize() runs generate_event_semaphores,
    # which splits multi-sem waits to fit the per-instruction ISA sync slots.
    nc = bacc.Bacc("TRN2", target_bir_lowering=False)

    i16 = mybir.dt.int16
    lrow = nc.dram_tensor("lrow", [1, E], fp32, kind="ExternalInput")
    prow = nc.dram_tensor("prow", [1, E], i16, kind="ExternalInput")
    # cols[:, 0:TILES] = head per row tile, cols[:, TILES:2*TILES] = parent
    cols = nc.dram_tensor("cols", [P, 2 * TILES], fp32, kind="ExternalInput")

    a_part = nc.dram_tensor("a_part", [ROWS_PER_CORE, E], fp32, kind="ExternalOutput")
    z_out = nc.dram_tensor("z_out", [P, TILES], fp32, kind="ExternalOutput")
    d_out = nc.dram_tensor("d_out", [P, TILES], fp32, kind="ExternalOutput")

    with TileContext(nc) as tc:
        with tc.tile_pool(name="singles", bufs=1) as singles:
            cols_t = singles.tile([P, 2 * TILES], fp32)
            nc.sync.dma_start(cols_t[:], cols[:, :])

            # Replicate logits (f32) and parent (int16) across all 128
            # partitions with stride-0-source broadcast DMAs, split into
            # quarters so the first tile's masked sums start after only a
            # quarter of the broadcast has landed; exp overlaps the DMAs.
            # Decreasing chunk sizes: the last chunk gates tile 0's final
            # masked sum, so keep it short.
            CHUNKS = [2048, 1024, 1024, 1024, 1024, 1024, 768, 256]
            NQ = len(CHUNKS)
            ex_b_t = singles.tile([P, E], fp32)
            par_b_t = singles.tile([P, E], i16)
            ex_b = ex_b_t[:]
            par_b = par_b_t[:]
            qslices = []
            off = 0
            for w in CHUNKS:
                qslices.append(slice(off, off + w))
                off += w
            for qs in qslices:
                w = qs.stop - qs.start
                nc.sync.dma_start(ex_b[:, qs], lrow[:, qs].to_broadcast((P, w)))
                nc.sync.dma_start(par_b[:, qs], prow[:, qs].to_broadcast((P, w)))
                nc.scalar.activation(ex_b[:, qs], ex_b[:, qs], ACT.Exp)

            zt = singles.tile([P, TILES], fp32)
            dt_ = singles.tile([P, TILES], fp32)
            onecol = singles.tile([P, 1], fp32)
            halfcol = singles.tile([P, 1], fp32)
            nc.gpsimd.memset(onecol[:], 1.0)
            nc.gpsimd.memset(halfcol[:], 0.5)

            from concourse.tile_rust import add_dep_helper

            with (
                tc.tile_pool(name="work", bufs=3) as workp,
                tc.tile_pool(name="small", bufs=3) as smallp,
            ):
                prev_smalls = None
                for t in range(TILES):
                    hc = cols_t[:, t : t + 1]
                    pc = cols_t[:, TILES + t : TILES + t + 1]
                    work = workp.tile([P, E], fp32, tag="work")
                    ssum = smallp.tile([P, 1], fp32, tag="ssum")
                    # work = (par_b == head[p]) * ex_b ; ssum = row sums
                    if t == 0:
                        # Tile 0 per broadcast chunk: each chunk's STT runs
                        # while later chunks are still in flight.
                        qsums = []
                        for q, qs in enumerate(qslices):
                            sq = smallp.tile(
                                [P, 1], fp32, tag=f"ssq{q}", name=f"ssq{q}"
                            )
                            nc.vector.scalar_tensor_tensor(
                                out=work[:, qs],
                                in0=par_b[:, qs],
                                scalar=hc,
                                in1=ex_b[:, qs],
                                op0=alu.is_equal,
                                op1=alu.mult,
                                accum_out=sq[:],
                            )
                            qsums.append(sq)
                            # running sum: all but the last add happen while
                            # later broadcast chunks are still in flight
                            if q == 1:
                                acc = smallp.tile([P, 1], fp32, tag="qacc")
                                nc.vector.tensor_tensor(
                                    acc[:], qsums[0][:], sq[:], alu.add
                                )
                            elif q > 1:
                                last = q == NQ - 1
                                dst = ssum[:] if last else acc[:]
                                r = nc.vector.tensor_tensor(
                                    dst, acc[:], sq[:], alu.add
                                )
                                if last:
                                    stt = r
                    else:
                        stt = nc.vector.scalar_tensor_tensor(
                            out=work[:],
                            in0=par_b,
                            scalar=hc,
                            in1=ex_b,
                            op0=alu.is_equal,
                            op1=alu.mult,
                            accum_out=ssum[:],
                        )
                    # The previous tile's tiny per-row ops must precede this
                    # STT in the DVE stream, or the scheduler defers them and
                    # stalls the ACT scale + store pipeline.
                    if prev_smalls:
                        for ps in prev_smalls:
                            add_dep_helper(stt.ins, ps.ins, False, "smalls first")
                    zc = zt[:, t : t + 1]
                    # g = 1/(max(ssum,tiny)*1.5): on rows with children Z is
                    # exactly 1.5; childless-head rows are all-zero so their
                    # g value is irrelevant (finite).
                    w_ = smallp.tile([P, 1], fp32, tag="w")
                    s2 = nc.vector.tensor_scalar(
                        w_[:], ssum[:], 1e-30, 1.5, alu.max, alu.mult
                    )
                    g = smallp.tile([P, 1], fp32, tag="g")
                    s3 = nc.vector.reciprocal(g[:], w_[:])
                    # Z = (ssum > 0) + 0.5  (S is exactly 1 when head has kids)
                    s1 = nc.vector.tensor_scalar(
                        zc, ssum[:], 0.0, 0.5, alu.is_gt, alu.add
                    )
                    # diag correction (head != parent): 0.5/Z = 1/(2Z)
                    z2 = smallp.tile([P, 1], fp32, tag="z2")
                    s4 = nc.vector.tensor_scalar(z2[:], zc, 2.0, None, alu.mult)
                    rz2 = smallp.tile([P, 1], fp32, tag="rz2")
                    s5 = nc.vector.reciprocal(rz2[:], z2[:])
                    s6 = nc.vector.tensor_scalar(
                        dt_[:, t : t + 1], hc, pc, rz2[:], alu.not_equal, alu.mult
                    )
                    rows_sl = slice(t * P, (t + 1) * P)
                    prev_smalls = (s1, s2, s3, s4, s5, s6)
                    if t <= 1:
                        # Early tiles' scale+store gate the DMA pipeline fill:
                        # chunk them so each store starts as soon as its chunk
                        # is scaled on ACT, smallest chunk first.
                        widths = [1024, 1024, 2048, 2048, 2048] if t == 0 else [4096, 4096]
                        co = 0
                        for w in widths:
                            cs = slice(co, co + w)
                            co += w
                            nc.scalar.mul(work[:, cs], work[:, cs], g[:])
                            nc.sync.dma_start(a_part[rows_sl, cs], work[:, cs])
                    else:
                        # scale the row block by g on ACT, then store
                        nc.scalar.mul(work[:], work[:], g[:])
                        nc.sync.dma_start(a_part[rows_sl, :], work[:])

                nc.sync.dma_start(z_out[:, :], zt[:])
                nc.sync.dma_start(d_out[:, :], dt_[:])

    return nc


def _get_program():
    global _PROG
    if _PROG is None:
        _ensure_path()
        _PROG = _build_program()
        _PROG.finalize()  # Bacc passes: wait splitting, reg alloc, DCE
    return _PROG


def kernel(branch_logits, parent, head, _trace=False):
    _ensure_path()
    from concourse.bass_utils import run_bass_kernel_spmd

    nc = _get_program()

    logits = np.ascontiguousarray(
        np.asarray(branch_logits, dtype=np.float32).reshape(1, E)
    )
    par_i = np.asarray(parent).astype(np.int16)  # values < 4096: exact in i16
    head_i = np.asarray(head).astype(np.int16)
    prow_np = np.ascontiguousarray(par_i.reshape(1, E))

    in_maps = []
    for c in range(N_CORES):
        sl = slice(c * ROWS_PER_CORE, (c + 1) * ROWS_PER_CORE)
        # [P, TILES] with [p, t] = value at global row c*1024 + t*128 + p
        hc = head_i[sl].astype(np.float32).reshape(TILES, P).T
        pcl = par_i[sl].astype(np.float32).reshape(TILES, P).T
        in_maps.append(
            {
                "lrow": logits,
                "prow": prow_np,
                "cols": np.ascontiguousarray(np.concatenate([hc, pcl], axis=1)),
            }
        )

    res = run_bass_kernel_spmd(
        nc, in_maps, core_ids=list(range(N_CORES)), trace=_trace
    )

    A = np.concatenate([r["a_part"] for r in res.results], axis=0)
    z = np.concatenate([r["z_out"].T.reshape(-1) for r in res.results])
    d = np.concatenate([r["d_out"].T.reshape(-1) for r in res.results])

    idx = np.arange(E)
    A[idx, idx] += d  # (head != parent) rows: diagonal stay-weight 0.5/Z

    if _trace:
        kernel.last_exec_time_ns = res.exec_time_ns
    return A, z


# revision 54
# speedup vs baseline: 1.0021x; 1.0021x over previous
"""Trainium2 Bass kernel for nn_TreeVariationalPosterior (segment_reduce).

Computes, for E=8192 edges and N=4096 nodes:
    B = softmax of branch_logits within each parent group      [E]
    S = segment_sum(B, parent)  (1 for nodes with children)    [N]
    Z = max(0.5 + S[head], eps)                                [E]
    A[i,k] = B[k]/Z[i] where parent[k]==head[i], diag override [E,E]

Sharding: rows of A (edges i) are split across 8 NeuronCores, 1024 rows
per core, built as 8 tiles of [128 rows x 8192 cols].

Key identity used on-device: with ex = exp(logits) (softmax is shift
invariant, so the segment max subtraction is unnecessary for N(0,1)
logits), each output row block is

    A[i,:] = (parent[:]==head[i]) * ex[:] * g[i],
    g[i]   = 1/(ssum[head[i]] * Z[i]),

and ssum[head[i]] = sum_k (parent[k]==head[i]) * ex[k] is exactly the
row-sum of the masked tile.  A single DVE scalar_tensor_tensor per tile
produces both the masked row block AND (via accum_out) its row sums, so
no segment scatter, gather, or cross-core collective is needed.

The in-tile value at the diagonal position (column == global row i) is
(parent[i]==head[i])*ex[i]*g[i], which already equals the reference
diagonal whenever head[i]==parent[i] and is 0 otherwise; the device
emits the remaining (head!=parent) * 0.5/Z[i] correction as a small
per-row vector which the host adds at A[i,i] during unshard assembly.
"""

import numpy as np

E = 8192
N_CORES = 8
ROWS_PER_CORE = E // N_CORES  # 1024
P = 128
TILES = ROWS_PER_CORE // P  # 8


def _ensure_path():
    try:
        import concourse.bass  # noqa: F401
    except ImportError:
        import sys

        for p in ("/opt/trn_rl_repo", "/root/.axon_site/_ro/trn_rl_repo"):
            if p not in sys.path:
                sys.path.insert(0, p)


_PROG = None


def _build_program():
    import concourse.bacc as bacc
    import concourse.mybir as mybir
    from concourse.mybir import AluOpType as alu
    from concourse.tile import TileContext

    fp32 = mybir.dt.float32
    ACT = mybir.ActivationFunctionType

    # Bacc (not plain Bass): its finalize() runs generate_event_semaphores,
    # which splits multi-sem waits to fit the per-instruction ISA sync slots.
    nc = bacc.Bacc("TRN2", target_bir_lowering=False)

    i16 = mybir.dt.int16
    lrow = nc.dram_tensor("lrow", [1, E], fp32, kind="ExternalInput")
    prow = nc.dram_tensor("prow", [1, E], i16, kind="ExternalInput")
    # cols[:, 0:TILES] = head per row tile, cols[:, TILES:2*TILES] = parent
    cols = nc.dram_tensor("cols", [P, 2 * TILES], fp32, kind="ExternalInput")

    a_part = nc.dram_tensor("a_part", [ROWS_PER_CORE, E], fp32, kind="ExternalOutput")
    z_out = nc.dram_tensor("z_out", [P, TILES], fp32, kind="ExternalOutput")
    d_out = nc.dram_tensor("d_out", [P, TILES], fp32, kind="ExternalOutput")

    with TileContext(nc) as tc:
        with tc.tile_pool(name="singles", bufs=1) as singles:
            cols_t = singles.tile([P, 2 * TILES], fp32)
            nc.sync.dma_start(cols_t[:], cols[:, :])

            # Replicate logits (f32) and parent (int16) across all 128
            # partitions with stride-0-source broadcast DMAs, split into
            # quarters so the first tile's masked sums start after only a
            # quarter of the broadcast has landed; exp overlaps the DMAs.
            # Decreasing chunk sizes: the last chunk gates tile 0's final
            # masked sum, so keep it short.
            CHUNKS = [2048, 1024, 1024, 1024, 1024, 1024, 768, 256]
            NQ = len(CHUNKS)
            ex_b_t = singles.tile([P, E], fp32)
            par_b_t = singles.tile([P, E], i16)
            ex_b = ex_b_t[:]
            par_b = par_b_t[:]
            qslices = []
            off = 0
            for w in CHUNKS:
                qslices.append(slice(off, off + w))
                off += w
            for qs in qslices:
                w = qs.stop - qs.start
                nc.sync.dma_start(ex_b[:, qs], lrow[:, qs].to_broadcast((P, w)))
                nc.sync.dma_start(par_b[:, qs], prow[:, qs].to_broadcast((P, w)))
                nc.scalar.activation(ex_b[:, qs], ex_b[:, qs], ACT.Exp)

            zt = singles.tile([P, TILES], fp32)
            dt_ = singles.tile([P, TILES], fp32)
            onecol = singles.tile([P, 1], fp32)
            halfcol = singles.tile([P, 1], fp32)
            nc.gpsimd.memset(onecol[:], 1.0)
            nc.gpsimd.memset(halfcol[:], 0.5)

            from concourse.tile_rust import add_dep_helper

            with (
                tc.tile_pool(name="work", bufs=3) as workp,
                tc.tile_pool(name="small", bufs=3) as smallp,
            ):
                prev_smalls = None
                for t in range(TILES):
                    hc = cols_t[:, t : t + 1]
                    pc = cols_t[:, TILES + t : TILES + t + 1]
                    work = workp.tile([P, E], fp32, tag="work")
                    ssum = smallp.tile([P, 1], fp32, tag="ssum")
                    # work = (par_b == head[p]) * ex_b ; ssum = row sums
                    if t == 0:
                        # Tile 0 per broadcast chunk: each chunk's STT runs
                        # while later chunks are still in flight.
                        qsums = []
                        for q, qs in enumerate(qslices):
                            sq = smallp.tile(
                                [P, 1], fp32, tag=f"ssq{q}", name=f"ssq{q}"
                            )
                            nc.vector.scalar_tensor_tensor(
                                out=work[:, qs],
                                in0=par_b[:, qs],
                                scalar=hc,
                                in1=ex_b[:, qs],
                                op0=alu.is_equal,
                                op1=alu.mult,
                                accum_out=sq[:],
                            )
                            qsums.append(sq)
                            # running sum: all but the last add happen while
                            # later broadcast chunks are still in flight
                            if q == 1:
                                acc = smallp.tile([P, 1], fp32, tag="qacc")
                                nc.vector.tensor_tensor(
                                    acc[:], qsums[0][:], sq[:], alu.add
                                )
                            elif q > 1:
                                last = q == NQ - 1
                                dst = ssum[:] if last else acc[:]
                                r = nc.vector.tensor_tensor(
                                    dst, acc[:], sq[:], alu.add
                                )
                                if last:
                                    stt = r
                    else:
                        stt = nc.vector.scalar_tensor_tensor(
                            out=work[:],
                            in0=par_b,
                            scalar=hc,
                            in1=ex_b,
                            op0=alu.is_equal,
                            op1=alu.mult,
                            accum_out=ssum[:],
                        )
                    # The previous tile's tiny per-row ops must precede this
                    # STT in the DVE stream, or the scheduler defers them and
                    # stalls the ACT scale + store pipeline.
                    if prev_smalls:
                        for ps in prev_smalls:
                            add_dep_helper(stt.ins, ps.ins, False, "smalls first")
                    zc = zt[:, t : t + 1]
                    # g = 1/(max(ssum,tiny)*1.5): on rows with children Z is
                    # exactly 1.5; childless-head rows are all-zero so their
                    # g value is irrelevant (finite).
                    w_ = smallp.tile([P, 1], fp32, tag="w")
                    s2 = nc.vector.tensor_scalar(
                        w_[:], ssum[:], 1e-30, 1.5, alu.max, alu.mult
                    )
                    g = smallp.tile([P, 1], fp32, tag="g")
                    s3 = nc.vector.reciprocal(g[:], w_[:])
                    # Z = (ssum > 0) + 0.5  (S is exactly 1 when head has kids)
                    s1 = nc.vector.tensor_scalar(
                        zc, ssum[:], 0.0, 0.5, alu.is_gt, alu.add
                    )
                    # diag correction (head != parent): 0.5/Z = 1/(2Z)
                    z2 = smallp.tile([P, 1], fp32, tag="z2")
                    s4 = nc.vector.tensor_scalar(z2[:], zc, 2.0, None, alu.mult)
                    rz2 = smallp.tile([P, 1], fp32, tag="rz2")
                    s5 = nc.vector.reciprocal(rz2[:], z2[:])
                    s6 = nc.vector.tensor_scalar(
                        dt_[:, t : t + 1], hc, pc, rz2[:], alu.not_equal, alu.mult
                    )
                    rows_sl = slice(t * P, (t + 1) * P)
                    prev_smalls = (s1, s2, s3, s4, s5, s6)
                    if t <= 2:
                        # Early tiles' scale+store gate the DMA pipeline fill:
                        # chunk them so each store starts as soon as its chunk
                        # is scaled on ACT, smallest chunk first.
                        widths = (
                            [1024, 1024, 2048, 2048, 2048]
                            if t == 0
                            else [4096, 4096]
                        )
                        co = 0
                        for w in widths:
                            cs = slice(co, co + w)
                            co += w
                            nc.scalar.mul(work[:, cs], work[:, cs], g[:])
                            nc.sync.dma_start(a_part[rows_sl, cs], work[:, cs])
                    else:
                        # scale the row block by g on ACT, then store
                        nc.scalar.mul(work[:], work[:], g[:])
                        nc.sync.dma_start(a_part[rows_sl, :], work[:])

                nc.sync.dma_start(z_out[:, :], zt[:])
                nc.sync.dma_start(d_out[:, :], dt_[:])

    return nc


def _get_program():
    global _PROG
    if _PROG is None:
        _ensure_path()
        _PROG = _build_program()
        _PROG.finalize()  # Bacc passes: wait splitting, reg alloc, DCE
    return _PROG


def kernel(branch_logits, parent, head, _trace=False):
    _ensure_path()
    from concourse.bass_utils import run_bass_kernel_spmd

    nc = _get_program()

    logits = np.ascontiguousarray(
        np.asarray(branch_logits, dtype=np.float32).reshape(1, E)
    )
    par_i = np.asarray(parent).astype(np.int16)  # values < 4096: exact in i16
    head_i = np.asarray(head).astype(np.int16)
    prow_np = np.ascontiguousarray(par_i.reshape(1, E))

    in_maps = []
    for c in range(N_CORES):
        sl = slice(c * ROWS_PER_CORE, (c + 1) * ROWS_PER_CORE)
        # [P, TILES] with [p, t] = value at global row c*1024 + t*128 + p
        hc = head_i[sl].astype(np.float32).reshape(TILES, P).T
        pcl = par_i[sl].astype(np.float32).reshape(TILES, P).T
        in_maps.append(
            {
                "lrow": logits,
                "prow": prow_np,
                "cols": np.ascontiguousarray(np.concatenate([hc, pcl], axis=1)),
            }
        )

    res = run_bass_kernel_spmd(
        nc, in_maps, core_ids=list(range(N_CORES)), trace=_trace
    )

    A = np.concatenate([r["a_part"] for r in res.results], axis=0)
    z = np.concatenate([r["z_out"].T.reshape(-1) for r in res.results])
    d = np.concatenate([r["d_out"].T.reshape(-1) for r in res.results])

    idx = np.arange(E)
    A[idx, idx] += d  # (head != parent) rows: diagonal stay-weight 0.5/Z

    if _trace:
        kernel.last_exec_time_ns = res.exec_time_ns
    return A, z
